# revision 13
# baseline (speedup 1.0000x reference)
"""Trainium2 Bass kernel for nn_MultiHeadQueryInstanceDecoder.

Strategy: shard the point dimension N across 8 cores (sequence parallel).
Per core, stream 512-point chunks through the trunk (input_proj + point_head),
write point_embed + mask_logits shards directly, and keep flash-softmax
partials (sumexp + exp-weighted point sums) in PSUM accumulators. A single
small AllReduce combines the per-core partials; the tiny query-side MLP and
score head are computed replicated on every core.

Layout notes:
  - activations live as [point(part), d(free)] for row ops (LN / gelu / norm),
    and are PE-transposed to [d(part), point(free)] to feed matmuls
    (TensorE contracts over the partition axis).
  - matmul operands are fp16 (1 cyc/row on PE, same as bf16, better mantissa);
    the softmax/pooled path uses float32r (TF32) so exp values stay fp32.
  - biases enter matmuls as extra K-rows against a constant ones row.
  - softmax uses a fixed shift (max over heads of exp(logit_scale)), which is
    mathematically exact for softmax and avoids online-max bookkeeping.
"""

import sys
import os

for _p in ("/opt/trn_rl_repo", "/root/.axon_site/_ro/trn_rl_repo"):
    if os.path.isdir(_p) and _p not in sys.path:
        sys.path.append(_p)

import numpy as np

import concourse.bacc as bacc
import concourse.tile as tile
import concourse.mybir as mybir
from concourse.bass import ts
from concourse.masks import make_identity
from concourse.bass_utils import run_bass_kernel_spmd

F32 = mybir.dt.float32
F32R = mybir.dt.float32r
F16 = mybir.dt.float16
I32 = mybir.dt.int32
AF = mybir.ActivationFunctionType
ALU = mybir.AluOpType

C = 72          # in channels
D = 256         # hidden
Q = 128         # queries per head
G = 3           # heads
N_FULL = 262144
N_CORES = 8
T = 128         # points per tile
NT = 4          # tiles per chunk
CH = T * NT     # 512 points per chunk
EPS_LN = 1e-5
GQ = G * Q


def _rsqrt_newton(nc, pool, v_ap, out, eps):
    """out = 1/sqrt(v + eps) on DVE only (no ACT table): quake seed + 2 Newton.

    v_ap/out: fp32 APs of identical small shape (e.g. [128, 4])."""
    shp = list(out.shape)
    va = pool.tile(shp, F32, tag="rs_v")
    tmp = pool.tile(shp, F32, tag="rs_t")
    nc.vector.tensor_scalar(out=va[:], in0=v_ap, scalar1=float(eps), scalar2=None,
                            op0=ALU.add)
    # y0 = bittrick(v)
    nc.vector.tensor_scalar(out=tmp[:].bitcast(I32), in0=va[:].bitcast(I32),
                            scalar1=1, scalar2=None, op0=ALU.logical_shift_right)
    nc.vector.tensor_scalar(out=out.bitcast(I32), in0=tmp[:].bitcast(I32),
                            scalar1=-1, scalar2=0x5F3759DF, op0=ALU.mult, op1=ALU.add)
    for _ in range(2):
        # y = y * (1.5 - 0.5 * v * y * y)
        nc.vector.tensor_mul(out=tmp[:], in0=out, in1=out)
        nc.vector.tensor_mul(out=tmp[:], in0=tmp[:], in1=va[:])
        nc.vector.tensor_scalar(out=tmp[:], in0=tmp[:], scalar1=-0.5, scalar2=1.5,
                                op0=ALU.mult, op1=ALU.add)
        nc.vector.tensor_mul(out=out, in0=out, in1=tmp[:])


def build(n_chunks=64, n_cores=N_CORES, shift=10.0, ln_id=True, dbg=False):
    NL = n_chunks * CH
    nc = bacc.Bacc("TRN2", target_bir_lowering=False, debug=False,
                   num_devices=n_cores, name="mhqid")

    # ---- I/O ----
    px = nc.dram_tensor("point_feat", [NL, C], F32, kind="ExternalInput")
    din = {}
    for nm, shp in [
        ("ip_w1", [C, D]), ("ip_b1", [D]), ("ip_ln_g", [D]), ("ip_ln_b", [D]),
        ("ip_w2", [D, D]), ("ip_b2", [D]),
        ("ph_ln_g", [D]), ("ph_ln_b", [D]), ("ph_w1", [D, D]), ("ph_b1", [D]),
        ("ph_w2", [D, D]), ("ph_b2", [D]),
        ("qe", [G, Q, D]), ("qh_ln_g", [G, D]), ("qh_ln_b", [G, D]),
        ("qh_w1", [G, D, D]), ("qh_b1", [G, D]), ("qh_w2", [G, D, D]), ("qh_b2", [G, D]),
        ("sh_ln_g", [G, D]), ("sh_ln_b", [G, D]), ("sh_w1", [G, D, D]), ("sh_b1", [G, D]),
        ("sh_w2", [G, D, 1]), ("sh_b2", [G, 1]), ("logit_scale", [G]),
    ]:
        din[nm] = nc.dram_tensor(nm, shp, F32, kind="ExternalInput")

    if dbg:
        dbg_a1 = nc.dram_tensor("dbg_a1", [T, D], F32, kind="ExternalOutput")
        dbg_a1g = nc.dram_tensor("dbg_a1g", [T, D], F32, kind="ExternalOutput")
        dbg_h = nc.dram_tensor("dbg_h", [T, D], F32, kind="ExternalOutput")
        dbg_bn = nc.dram_tensor("dbg_bn", [T, D], F32, kind="ExternalOutput")
        dbg_cg = nc.dram_tensor("dbg_cg", [D, T], F32, kind="ExternalOutput")
        dbg_qms = nc.dram_tensor("dbg_qms", [Q, G, D], F32, kind="ExternalOutput")
        dbg_exp = nc.dram_tensor("dbg_exp", [T, GQ], F32, kind="ExternalOutput")
    pe_out = nc.dram_tensor("pe_out", [NL, D], F32, kind="ExternalOutput")
    ml_out = nc.dram_tensor("ml_out", [G, Q, NL], F32, kind="ExternalOutput")
    score_out = nc.dram_tensor("score_out", [G, Q], F32, kind="ExternalOutput")
    refined_out = nc.dram_tensor("refined_out", [G, Q, D], F32, kind="ExternalOutput")

    from contextlib import ExitStack
    with tile.TileContext(nc) as tc, ExitStack() as ctx:
        wp = ctx.enter_context(tc.tile_pool(name="wp", bufs=1))
        s2 = ctx.enter_context(tc.tile_pool(name="s2", bufs=2))
        s3 = ctx.enter_context(tc.tile_pool(name="s3", bufs=3))
        stp = ctx.enter_context(tc.tile_pool(name="stp", bufs=3))
        pp = ctx.enter_context(tc.tile_pool(name="pp", bufs=3, space="PSUM"))
        hp = ctx.enter_context(tc.tile_pool(name="hp", bufs=1, space="PSUM"))
        ap = ctx.enter_context(tc.tile_pool(name="ap", bufs=1, space="PSUM"))
        dp = ctx.enter_context(tc.tile_pool(name="dp", bufs=1, space="DRAM"))

        # ---------------- prologue: constants & weights ----------------
        id16 = wp.tile([128, 128], F16)
        make_identity(nc, id16[:])
        id32 = wp.tile([128, 128], F32)
        make_identity(nc, id32[:])
        ones16 = wp.tile([1, CH], F16)
        nc.gpsimd.memset(ones16[:], 1.0)
        ones32 = wp.tile([1, 128], F32)
        nc.gpsimd.memset(ones32[:], 1.0)
        c_eps = wp.tile([128, 1], F32)
        nc.gpsimd.memset(c_eps[:], EPS_LN)
        c_eps24 = wp.tile([128, 1], F32)
        nc.gpsimd.memset(c_eps24[:], 1e-24)
        c_shift = wp.tile([128, 1], F32)
        nc.gpsimd.memset(c_shift[:], -float(shift))

        # trunk weights, fp16. ip_w1 lives in rows 0:72 of a 97-row tile with
        # the bias in row 96 (partition bases must be 32-aligned); rows 72:96
        # are zeroed on both the weight and activation side.
        w1b = wp.tile([97, D], F16)
        nc.gpsimd.memset(w1b[:], 0.0)
        nc.gpsimd.dma_start(out=w1b[0:C, :], in_=din["ip_w1"][:, :])
        nc.gpsimd.dma_start(out=w1b[96:97, :], in_=din["ip_b1"][None, :])
        w2 = wp.tile([128, 2, D], F16)
        nc.gpsimd.dma_start(out=w2[:], in_=din["ip_w2"].rearrange("(k p) d -> p k d", p=128))
        b2row = wp.tile([1, D], F16)
        nc.gpsimd.dma_start(out=b2row[:], in_=din["ip_b2"][None, :])
        p1w = wp.tile([128, 2, D], F16)
        nc.gpsimd.dma_start(out=p1w[:], in_=din["ph_w1"].rearrange("(k p) d -> p k d", p=128))
        p1bc = wp.tile([128, 2], F32)
        nc.gpsimd.dma_start(out=p1bc[:], in_=din["ph_b1"].rearrange("(k p) -> p k", p=128))
        p2w = wp.tile([128, 2, D], F16)
        nc.gpsimd.dma_start(out=p2w[:], in_=din["ph_w2"].rearrange("(k p) d -> p k d", p=128))
        p2brow = wp.tile([1, D], F16)
        nc.gpsimd.dma_start(out=p2brow[:], in_=din["ph_b2"][None, :])

        # optional general LN gamma/beta for trunk (broadcast across partitions)
        if not ln_id:
            ipg_bc = wp.tile([128, D], F32)
            ipb_bc = wp.tile([128, D], F32)
            phg_bc = wp.tile([128, D], F32)
            phb_bc = wp.tile([128, D], F32)
            for t_, src in ((ipg_bc, "ip_ln_g"), (ipb_bc, "ip_ln_b"),
                            (phg_bc, "ph_ln_g"), (phb_bc, "ph_ln_b")):
                nc.gpsimd.dma_start(out=t_[:], in_=din[src][None, :].broadcast_to([128, D]))

        # query-side weights (fp32; prologue/epilogue only)
        qw1 = wp.tile([128, G, 2, D], F32)
        nc.sync.dma_start(out=qw1[:], in_=din["qh_w1"].rearrange("g (k p) d -> p g k d", p=128))
        qw2 = wp.tile([128, G, 2, D], F32)
        nc.sync.dma_start(out=qw2[:], in_=din["qh_w2"].rearrange("g (k p) d -> p g k d", p=128))
        qb1r = wp.tile([1, G, D], F32)
        nc.sync.dma_start(out=qb1r[:], in_=din["qh_b1"][None, :, :])
        qb2r = wp.tile([1, G, D], F32)
        nc.sync.dma_start(out=qb2r[:], in_=din["qh_b2"][None, :, :])
        sw1 = wp.tile([128, G, 2, D], F32)
        nc.sync.dma_start(out=sw1[:], in_=din["sh_w1"].rearrange("g (k p) d -> p g k d", p=128))
        sw2c = wp.tile([128, G, 2], F32)
        nc.sync.dma_start(out=sw2c[:], in_=din["sh_w2"].rearrange("g (k p) o -> p (g k o)", p=128))
        sb1r = wp.tile([1, G, D], F32)
        nc.sync.dma_start(out=sb1r[:], in_=din["sh_b1"][None, :, :])
        sb2r = wp.tile([1, G], F32)
        nc.sync.dma_start(out=sb2r[:], in_=din["sh_b2"][None, :, 0])
        qe_sb = wp.tile([128, G, D], F32)
        nc.sync.dma_start(out=qe_sb[:], in_=din["qe"].rearrange("g q d -> q g d"))
        qhg_bc = wp.tile([128, G, D], F32)
        nc.sync.dma_start(out=qhg_bc[:], in_=din["qh_ln_g"][None, :, :].broadcast_to([128, G, D]))
        qhb_bc = wp.tile([128, G, D], F32)
        nc.sync.dma_start(out=qhb_bc[:], in_=din["qh_ln_b"][None, :, :].broadcast_to([128, G, D]))
        shg_bc = wp.tile([128, G, D], F32)
        nc.sync.dma_start(out=shg_bc[:], in_=din["sh_ln_g"][None, :, :].broadcast_to([128, G, D]))
        shb_bc = wp.tile([128, G, D], F32)
        nc.sync.dma_start(out=shb_bc[:], in_=din["sh_ln_b"][None, :, :].broadcast_to([128, G, D]))
        ls_sb = wp.tile([1, G], F32)
        nc.sync.dma_start(out=ls_sb[:], in_=din["logit_scale"][None, :])

        # ------------- query MLP (replicated, fp32) -------------
        s_row = wp.tile([1, G], F32)
        nc.scalar.activation(out=s_row[:], in_=ls_sb[:], func=AF.Exp)
        s128 = wp.tile([128, G], F32)
        nc.gpsimd.partition_broadcast(s128[:], s_row[:])

        qemb = wp.tile([128, G, D], F32)

        def f32_ln(src_ap, g_bc, b_bc, gi, dst_ap):
            """dst = LN(src) * g + b, fp32, row ops. src/dst: [128, D] APs."""
            st = stp.tile([128, 6], F32, tag="qst")
            mv = stp.tile([128, 2], F32, tag="qmv")
            nc.vector.bn_stats(out=st[:], in_=src_ap)
            nc.vector.bn_aggr(out=mv[:], in_=st[:])
            rs_ = stp.tile([128, 1], F32, tag="qrs")
            _rsqrt_newton(nc, stp, mv[:, 1:2], rs_[:], EPS_LN)
            nm = stp.tile([128, 1], F32, tag="qnm")
            nc.vector.scalar_tensor_tensor(out=nm[:], in0=mv[:, 0:1], scalar=-1.0,
                                           in1=rs_[:], op0=ALU.mult, op1=ALU.mult)
            nc.scalar.activation(out=dst_ap, in_=src_ap, func=AF.Identity,
                                 bias=nm[:], scale=rs_[:])
            nc.vector.tensor_mul(out=dst_ap, in0=dst_ap, in1=g_bc[:, gi, :])
            nc.vector.tensor_add(out=dst_ap, in0=dst_ap, in1=b_bc[:, gi, :])

        def f32_transpose(src_ap, tag):
            """[128, D] fp32 -> list of two [128, 128] sbuf tiles (transposed)."""
            outs = []
            for k in range(2):
                pst = pp.tile([128, 512], F32, tag="ps")
                nc.tensor.transpose(pst[0:128, 0:128], src_ap[:, ts(k, 128)], id32[:])
                sb = s3.tile([128, 128], F32, tag=tag + str(k))
                nc.vector.tensor_copy(out=sb[:], in_=pst[0:128, 0:128])
                outs.append(sb)
            return outs

        for g in range(G):
            qn = s3.tile([128, D], F32, tag="qn")
            f32_ln(qe_sb[:, g, :], qhg_bc, qhb_bc, g, qn[:])
            qnT = f32_transpose(qn[:], "qnT")
            ps1 = pp.tile([128, 512], F32, tag="ps")
            for k in range(2):
                nc.tensor.matmul(ps1[:, 0:D], lhsT=qnT[k][:], rhs=qw1[:, g, k, :],
                                 start=(k == 0), stop=False)
            nc.tensor.matmul(ps1[:, 0:D], lhsT=ones32[:, 0:128], rhs=qb1r[:, g, :],
                             start=False, stop=True)
            q1 = s3.tile([128, D], F32, tag="q1")
            nc.scalar.activation(out=q1[:], in_=ps1[:, 0:D], func=AF.Gelu)
            q1T = f32_transpose(q1[:], "q1T")
            ps2 = pp.tile([128, 512], F32, tag="ps")
            for k in range(2):
                nc.tensor.matmul(ps2[:, 0:D], lhsT=q1T[k][:], rhs=qw2[:, g, k, :],
                                 start=(k == 0), stop=False)
            nc.tensor.matmul(ps2[:, 0:D], lhsT=ones32[:, 0:128], rhs=qb2r[:, g, :],
                             start=False, stop=True)
            nc.vector.scalar_tensor_tensor(out=qemb[:, g, :], in0=ps2[:, 0:D],
                                           scalar=1.0, in1=qe_sb[:, g, :],
                                           op0=ALU.mult, op1=ALU.add)

        # qmask scaled: qms = qemb * rsqrt(sum(qemb^2)) * exp(logit_scale)
        qssq = wp.tile([128, G], F32)
        junkq = s3.tile([128, D], F32, tag="junkq")
        for g in range(G):
            nc.scalar.activation(out=junkq[:], in_=qemb[:, g, :], func=AF.Square,
                                 accum_out=qssq[:, g:g + 1])
        qrn = wp.tile([128, G], F32)
        _rsqrt_newton(nc, stp, qssq[:], qrn[:], 1e-24)
        qrs = wp.tile([128, G], F32)
        nc.vector.tensor_mul(out=qrs[:], in0=qrn[:], in1=s128[:])
        qms = wp.tile([128, G, D], F16)
        for g in range(G):
            nc.vector.tensor_scalar(out=qms[:, g, :], in0=qemb[:, g, :],
                                    scalar1=qrs[:, g:g + 1], scalar2=None, op0=ALU.mult)
        if dbg:
            _dq = wp.tile([128, G, D], F32, tag="dbgq")
            nc.vector.tensor_copy(out=_dq[:], in_=qms[:])
            nc.sync.dma_start(out=dbg_qms[:, :, :], in_=_dq[:])
        # qmsT[k][d_in_chunk, g*128+q]
        qmsT = []
        for k in range(2):
            qt_ = wp.tile([128, GQ], F16, tag=f"qmsT{k}")
            for g in range(G):
                pst = pp.tile([128, 512], F16, tag="ps")
                nc.tensor.transpose(pst[0:128, 0:128], qms[:, g, ts(k, 128)], id16[:])
                nc.vector.tensor_copy(out=qt_[:, ts(g, 128)], in_=pst[0:128, 0:128])
            qmsT.append(qt_)

        # ------------- main loop over point chunks -------------
        acc = ap.tile([128, 3 * 512], F32)  # pooled sums + sumexp per head

        for c in range(n_chunks):
            c0 = c * CH
            xf32 = s3.tile([128, NT, C], F32, tag="xf32")
            nc.sync.dma_start(out=xf32[:],
                              in_=px[c0:c0 + CH, :].rearrange("(j p) m -> p j m", p=128))
            xf16 = s3.tile([128, NT, C], F16, tag="xf16")
            nc.gpsimd.tensor_copy(out=xf16[:], in_=xf32[:])

            # xT with a ones row at partition 96 -> [97, CH]
            xTb = s2.tile([97, CH], F16, tag="xTb")
            nc.gpsimd.memset(xTb[:], 0.0)
            nc.gpsimd.memset(xTb[96:97, :], 1.0)
            ps_t0 = pp.tile([128, 512], F16, tag="ps")
            for j in range(NT):
                nc.tensor.transpose(ps_t0[0:C, ts(j, 128)], xf16[:, j, :], id16[:])
            nc.vector.tensor_copy(out=xTb[0:C, :], in_=ps_t0[0:C, :])

            # mm1 + LN1 stats (a1 packed two tiles per psum slot)
            st1 = stp.tile([128, NT, 6], F32, tag="st1")
            mv1 = stp.tile([128, NT, 2], F32, tag="mv1")
            a1ps = []
            for u in range(2):
                a1u = pp.tile([128, 512], F32, tag="ps")
                for v in range(2):
                    t = 2 * u + v
                    nc.tensor.matmul(a1u[:, ts(v, D)], lhsT=xTb[:, ts(t, 128)],
                                     rhs=w1b[:], start=True, stop=True)
                    nc.vector.bn_stats(out=st1[:, t, :], in_=a1u[:, ts(v, D)])
                    if dbg and c == 0 and t == 0:
                        _d = s3.tile([T, D], F32, tag="dbgt")
                        nc.vector.tensor_copy(out=_d[:], in_=a1u[:, ts(v, D)])
                        nc.sync.dma_start(out=dbg_a1[:, :], in_=_d[:])
                a1ps.append(a1u)
            for t in range(NT):
                nc.vector.bn_aggr(out=mv1[:, t, :], in_=st1[:, t, :])
            # rstd1 via Ln/Exp on ACT (rides the exp table phase)
            r1 = stp.tile([128, NT], F32, tag="r1")
            l1 = stp.tile([128, NT], F32, tag="l1")
            nc.scalar.activation(out=l1[:], in_=mv1[:, :, 1], func=AF.Ln, bias=c_eps[:])
            nc.scalar.activation(out=r1[:], in_=l1[:], func=AF.Exp, scale=-0.5)
            nm1 = stp.tile([128, NT], F32, tag="nm1")
            nc.vector.scalar_tensor_tensor(out=nm1[:], in0=mv1[:, :, 0], scalar=-1.0,
                                           in1=r1[:], op0=ALU.mult, op1=ALU.mult)

            # LN1 apply + gelu (fp16 out)
            a1g = s2.tile([128, NT, D], F16, tag="a1g")
            for t in range(NT):
                src = a1ps[t // 2][:, ts(t % 2, D)]
                if ln_id:
                    nc.scalar.activation(out=a1g[:, t, :], in_=src, func=AF.Gelu,
                                         bias=nm1[:, t:t + 1], scale=r1[:, t:t + 1])
                else:
                    tmp = s3.tile([128, D], F32, tag="lntmp")
                    nc.scalar.activation(out=tmp[:], in_=src, func=AF.Identity,
                                         bias=nm1[:, t:t + 1], scale=r1[:, t:t + 1])
                    nc.vector.tensor_mul(out=tmp[:], in0=tmp[:], in1=ipg_bc[:])
                    nc.vector.tensor_add(out=tmp[:], in0=tmp[:], in1=ipb_bc[:])
                    nc.scalar.activation(out=a1g[:, t, :], in_=tmp[:], func=AF.Gelu)

            if dbg and c == 0:
                _d = s3.tile([T, D], F32, tag="dbgt")
                nc.vector.tensor_copy(out=_d[:], in_=a1g[:, 0, :])
                nc.sync.dma_start(out=dbg_a1g[:, :], in_=_d[:])

            # t1: a1g -> a1gT fp16 [2][128, CH]
            a1gT = []
            for k in range(2):
                psk = pp.tile([128, 512], F16, tag="ps")
                for t in range(NT):
                    nc.tensor.transpose(psk[0:128, ts(t, 128)],
                                        a1g[:, t, ts(k, 128)], id16[:])
                sb = s2.tile([128, CH], F16, tag=f"a1gT{k}")
                nc.scalar.activation(out=sb[:], in_=psk[:], func=AF.Copy)
                a1gT.append(sb)

            # mm2 -> h (psum, lives until mm4 accumulates the residual)
            h_ps = hp.tile([128, NT, D], F32, tag="h")
            # NOTE: PSUM start=True clears the whole 2KB bank region, so only
            # the first matmul touching each bank of h may set start=True; the
            # bank-sibling tile's first write then overwrites via pending-zero.
            for t in range(NT):
                for k in range(2):
                    nc.tensor.matmul(h_ps[:, t, :], lhsT=a1gT[k][:, ts(t, 128)],
                                     rhs=w2[:, k, :],
                                     start=(k == 0 and t % 2 == 0), stop=False,
                                     skip_group_check=True)
                nc.tensor.matmul(h_ps[:, t, :], lhsT=ones16[:, ts(t, 128)],
                                 rhs=b2row[:], start=False, stop=False,
                                 skip_group_check=True)

            if dbg and c == 0:
                _d = s3.tile([T, D], F32, tag="dbgt")
                nc.vector.tensor_copy(out=_d[:], in_=h_ps[:, 0, :])
                nc.sync.dma_start(out=dbg_h[:, :], in_=_d[:])

            # LN2 stats
            st2 = stp.tile([128, NT, 6], F32, tag="st2")
            mv2 = stp.tile([128, NT, 2], F32, tag="mv2")
            for t in range(NT):
                nc.vector.bn_stats(out=st2[:, t, :], in_=h_ps[:, t, :])
                nc.vector.bn_aggr(out=mv2[:, t, :], in_=st2[:, t, :])
            r2 = stp.tile([128, NT], F32, tag="r2")
            _rsqrt_newton(nc, stp, mv2[:, :, 1], r2[:], EPS_LN)
            nm2 = stp.tile([128, NT], F32, tag="nm2")
            nc.vector.scalar_tensor_tensor(out=nm2[:], in0=mv2[:, :, 0], scalar=-1.0,
                                           in1=r2[:], op0=ALU.mult, op1=ALU.mult)

            bn = s2.tile([128, NT, D], F16, tag="bn")
            for t in range(NT):
                if ln_id:
                    nc.scalar.activation(out=bn[:, t, :], in_=h_ps[:, t, :],
                                         func=AF.Identity,
                                         bias=nm2[:, t:t + 1], scale=r2[:, t:t + 1])
                else:
                    tmp = s3.tile([128, D], F32, tag="lntmp")
                    nc.scalar.activation(out=tmp[:], in_=h_ps[:, t, :], func=AF.Identity,
                                         bias=nm2[:, t:t + 1], scale=r2[:, t:t + 1])
                    nc.vector.tensor_mul(out=tmp[:], in0=tmp[:], in1=phg_bc[:])
                    nc.vector.tensor_add(out=bn[:, t, :], in0=tmp[:], in1=phb_bc[:])

            if dbg and c == 0:
                _d = s3.tile([T, D], F32, tag="dbgt")
                nc.vector.tensor_copy(out=_d[:], in_=bn[:, 0, :])
                nc.sync.dma_start(out=dbg_bn[:, :], in_=_d[:])

            # t2: bn -> bnT
            bnT = []
            for k in range(2):
                psk = pp.tile([128, 512], F16, tag="ps")
                for t in range(NT):
                    nc.tensor.transpose(psk[0:128, ts(t, 128)],
                                        bn[:, t, ts(k, 128)], id16[:])
                sb = s2.tile([128, CH], F16, tag=f"bnT{k}")
                nc.scalar.activation(out=sb[:], in_=psk[:], func=AF.Copy)
                bnT.append(sb)

            # mm3 (chunked, transposed out) + gelu evict with per-partition bias
            cgT = []
            for dc in range(2):
                psk = pp.tile([128, 512], F32, tag="ps")
                for k in range(2):
                    nc.tensor.matmul(psk[:, :], lhsT=p1w[:, k, ts(dc, 128)],
                                     rhs=bnT[k][:], start=(k == 0), stop=(k == 1))
                sb = s2.tile([128, CH], F16, tag=f"cgT{dc}")
                nc.scalar.activation(out=sb[:], in_=psk[:], func=AF.Gelu,
                                     bias=p1bc[:, dc:dc + 1], scale=1.0)
                cgT.append(sb)

            if dbg and c == 0:
                for _k in range(2):
                    _d = s3.tile([T, T], F32, tag="dbgt2")
                    nc.vector.tensor_copy(out=_d[:], in_=cgT[_k][:, 0:128])
                    nc.sync.dma_start(out=dbg_cg[ts(_k, 128), 0:128], in_=_d[:])

            # mm4 accumulates the point-head delta onto h in-place -> pe
            for t in range(NT):
                for k in range(2):
                    nc.tensor.matmul(h_ps[:, t, :], lhsT=cgT[k][:, ts(t, 128)],
                                     rhs=p2w[:, k, :], start=False, stop=False,
                                     skip_group_check=True)
                nc.tensor.matmul(h_ps[:, t, :], lhsT=ones16[:, ts(t, 128)],
                                 rhs=p2brow[:], start=False, stop=(t % 2 == 1),
                                 skip_group_check=True)

            # pe evict: fp32 copy for DMA/norm/pmask, fp16 copy (with a ones
            # column for the sumexp) as the pooled-matmul values operand
            pe_sb = s3.tile([128, NT, D], F32, tag="pe")
            nc.vector.tensor_copy(out=pe_sb[:, :, :], in_=h_ps[:, :, :])
            pe16 = s3.tile([128, NT, D + 1], F16, tag="pe16")
            nc.vector.tensor_copy(out=pe16[:, :, 0:D], in_=h_ps[:, :, :])
            nc.gpsimd.memset(pe16[:, :, D], 1.0)
            nc.sync.dma_start(
                out=pe_out[c0:c0 + CH, :].rearrange("(j p) d -> p j d", p=128),
                in_=pe_sb[:, :, :])

            # point norm: rn = rsqrt(sum(pe^2)); gpsimd computes the squares sum
            ssq = stp.tile([128, NT], F32, tag="ssq")
            junk = s3.tile([128, D], F32, tag="junk")
            for t in range(NT):
                nc.vector.scalar_tensor_tensor(out=junk[:], in0=pe_sb[:, t, :],
                                               scalar=1.0, in1=pe_sb[:, t, :],
                                               op0=ALU.mult, op1=ALU.mult,
                                               accum_out=ssq[:, t:t + 1])
            lt = stp.tile([128, NT], F32, tag="lt")
            rn = stp.tile([128, NT], F32, tag="rn")
            nc.scalar.activation(out=lt[:], in_=ssq[:], func=AF.Ln, bias=c_eps24[:])
            nc.scalar.activation(out=rn[:], in_=lt[:], func=AF.Exp, scale=-0.5)

            pmask = s2.tile([128, NT, D], F16, tag="pmask")
            for t in range(NT):
                nc.vector.tensor_scalar(out=pmask[:, t, :], in0=pe_sb[:, t, :],
                                        scalar1=rn[:, t:t + 1], scalar2=None,
                                        op0=ALU.mult)

            # t4: pmask -> pmT
            pmT = []
            for k in range(2):
                psk = pp.tile([128, 512], F16, tag="ps")
                for t in range(NT):
                    nc.tensor.transpose(psk[0:128, ts(t, 128)],
                                        pmask[:, t, ts(k, 128)], id16[:])
                sb = s2.tile([128, CH], F16, tag=f"pmT{k}")
                nc.vector.tensor_copy(out=sb[:], in_=psk[:])
                pmT.append(sb)

            # qt logits: [q, n] per head -> mask_logits output
            for g in range(G):
                qtp = pp.tile([128, 512], F32, tag="ps")
                for k in range(2):
                    nc.tensor.matmul(qtp[:, :], lhsT=qmsT[k][:, ts(g, 128)],
                                     rhs=pmT[k][:], start=(k == 0), stop=(k == 1))
                qsb = s3.tile([128, CH], F32, tag="qt")
                nc.vector.tensor_copy(out=qsb[:], in_=qtp[:])
                nc.sync.dma_start(out=ml_out[g, :, c0:c0 + CH], in_=qsb[:])

            # nt logits + exp + pooled accumulation
            exp_sb = s3.tile([128, NT, GQ], F16, tag="exp")
            for t in range(NT):
                ntp = pp.tile([128, 512], F32, tag="ps")
                for k in range(2):
                    nc.tensor.matmul(ntp[:, 0:GQ], lhsT=pmT[k][:, ts(t, 128)],
                                     rhs=qmsT[k][:], start=(k == 0), stop=(k == 1))
                nc.scalar.activation(out=exp_sb[:, t, :], in_=ntp[:, 0:GQ],
                                     func=AF.Exp, bias=c_shift[:])
                if dbg and c == 0 and t == 0:
                    _d = s3.tile([T, GQ], F32, tag="dbgt3")
                    nc.vector.tensor_copy(out=_d[:], in_=exp_sb[:, 0, :])
                    nc.sync.dma_start(out=dbg_exp[:, :], in_=_d[:])
                for g in range(G):
                    nc.tensor.matmul(
                        acc[:, g * 512:g * 512 + D + 1],
                        lhsT=exp_sb[:, t, ts(g, 128)],
                        rhs=pe16[:, t, :],
                        start=(c == 0 and t == 0),
                        stop=(c == n_chunks - 1 and t == NT - 1),
                        skip_group_check=True)

        # ------------- epilogue: AllReduce + refined + score head -------------
        accs = wp.tile([128, G, D + 1], F32)
        for g in range(G):
            nc.vector.tensor_copy(out=accs[:, g, :], in_=acc[:, g * 512:g * 512 + D + 1])
        cc_in = dp.tile([128, G * (D + 1)], F32)
        cc_out = dp.tile([128, G * (D + 1)], F32)
        nc.sync.dma_start(out=cc_in[:], in_=accs[:].rearrange("p g d -> p (g d)"))
        if n_cores > 1:
            nc.gpsimd.collective_compute(
                "AllReduce", ALU.add,
                replica_groups=[list(range(n_cores))],
                ins=[cc_in.opt()], outs=[cc_out.opt()])
            asum_src = cc_out
        else:
            asum_src = cc_in
        asum = wp.tile([128, G, D + 1], F32)
        nc.sync.dma_start(out=asum[:].rearrange("p g d -> p (g d)"), in_=asum_src[:])

        rden = wp.tile([128, G], F32)
        nc.vector.reciprocal(out=rden[:], in_=asum[:, :, D])
        ref_sb = wp.tile([128, G, D], F32)
        for g in range(G):
            nc.vector.tensor_scalar(out=ref_sb[:, g, :], in0=asum[:, g, 0:D],
                                    scalar1=rden[:, g:g + 1], scalar2=None, op0=ALU.mult)
            nc.vector.tensor_add(out=ref_sb[:, g, :], in0=ref_sb[:, g, :],
                                 in1=qemb[:, g, :])
        nc.sync.dma_start(out=refined_out.rearrange("g q d -> q g d"), in_=ref_sb[:])

        score_sb = wp.tile([128, G], F32)
        for g in range(G):
            xn = s3.tile([128, D], F32, tag="xn")
            f32_ln(ref_sb[:, g, :], shg_bc, shb_bc, g, xn[:])
            xnT = f32_transpose(xn[:], "xnT")
            ps1 = pp.tile([128, 512], F32, tag="ps")
            for k in range(2):
                nc.tensor.matmul(ps1[:, 0:D], lhsT=xnT[k][:], rhs=sw1[:, g, k, :],
                                 start=(k == 0), stop=False)
            nc.tensor.matmul(ps1[:, 0:D], lhsT=ones32[:, 0:128], rhs=sb1r[:, g, :],
                             start=False, stop=True)
            s1_ = s3.tile([128, D], F32, tag="s1")
            nc.scalar.activation(out=s1_[:], in_=ps1[:, 0:D], func=AF.Gelu)
            s1T = f32_transpose(s1_[:], "s1T")
            ps2 = pp.tile([128, 512], F32, tag="ps")
            for k in range(2):
                nc.tensor.matmul(ps2[:, 0:1], lhsT=s1T[k][:], rhs=sw2c[:, g, k:k + 1],
                                 start=(k == 0), stop=False)
            nc.tensor.matmul(ps2[:, 0:1], lhsT=ones32[:, 0:128], rhs=sb2r[:, g:g + 1],
                             start=False, stop=True)
            nc.vector.tensor_copy(out=score_sb[:, g:g + 1], in_=ps2[:, 0:1])
        nc.sync.dma_start(out=score_out.rearrange("g q -> q g"), in_=score_sb[:])

    nc.finalize()
    return nc


_CACHE = {}


def _get_nc(n_chunks, n_cores, shift, ln_id):
    key = (n_chunks, n_cores, round(shift, 6), ln_id)
    if key not in _CACHE:
        _CACHE[key] = build(n_chunks=n_chunks, n_cores=n_cores, shift=shift,
                            ln_id=ln_id)
    return _CACHE[key]


def kernel(**inputs):
    inputs = {k: np.ascontiguousarray(np.asarray(v, dtype=np.float32))
              for k, v in inputs.items()}
    pf = inputs["point_feat"]
    n = pf.shape[0]
    assert n % (N_CORES * CH) == 0, n
    n_chunks = n // (N_CORES * CH)
    nl = n_chunks * CH

    shift = float(np.exp(inputs["logit_scale"]).max())
    ln_id = bool(
        np.all(inputs["ip_ln_g"] == 1.0) and np.all(inputs["ip_ln_b"] == 0.0)
        and np.all(inputs["ph_ln_g"] == 1.0) and np.all(inputs["ph_ln_b"] == 0.0))

    nc = _get_nc(n_chunks, N_CORES, shift, ln_id)

    weights = {k: v for k, v in inputs.items() if k != "point_feat"}
    in_maps = []
    for i in range(N_CORES):
        m = dict(weights)
        m["point_feat"] = np.ascontiguousarray(pf[i * nl:(i + 1) * nl])
        in_maps.append(m)

    res = run_bass_kernel_spmd(nc, in_maps, core_ids=list(range(N_CORES)))
    rs = res.results
    point_embed = np.concatenate([rs[i]["pe_out"] for i in range(N_CORES)], axis=0)
    mask_logits = np.concatenate([rs[i]["ml_out"] for i in range(N_CORES)], axis=2)
    score_logits = rs[0]["score_out"]
    refined = rs[0]["refined_out"]
    return point_embed, mask_logits, score_logits, refined
